# revision 12
# baseline (speedup 1.0000x reference)
"""Trainium2 Bass kernel for nn_BatchProgramCC (tree-CNN + BiGRU program-pair
classifier). Self-contained: hardcodes shapes/sharding; builds+runs an 8-core
SPMD Bass program via run_bass_kernel_spmd.

Sharding: data-parallel over B (8 programs/core); embedding table + all
weights replicated. Per core, per side: indirect-DMA gather of 16384 emb rows
(tree order, k-outer), DMA-transpose to [E, tok], fixed-topology subtree-sum
(15 vector adds, moved before W_c by linearity), W_c matmul + per-node max +
relu -> enc, input-side GRU matmuls, then a fully unrolled 128-step
bidirectional GRU and max-pool; final fc + 2-class softmax on device.
"""
import os
import numpy as np
import ml_dtypes

# ---- problem constants (hardcoded per contract) ----
B, S, K = 64, 128, 16
MAX_DEPTH = 5
V, E, H = 50000, 128, 100
NCORES = 8
BL = B // NCORES            # programs per core = 8
NT = BL * S * K             # tokens per core per side = 16384
NTREE = BL * S              # trees per core = 1024
NCALL = NT // 128           # indirect gather calls per side = 128

# fixed binary-tree topology (matches reference._tree_structure)
_LOCAL_PARENT = np.array([0] + [(i - 1) // 2 for i in range(1, K)], dtype=np.int64)
_LOCAL_LEVEL = np.floor(np.log2(np.arange(K) + 1)).astype(np.int64)
# child-sum edge schedule, bottom-up: (parent, child) pairs in dependency order
_EDGES = [(7, 15),
          (3, 7), (3, 8), (4, 9), (4, 10), (5, 11), (5, 12), (6, 13), (6, 14),
          (1, 3), (1, 4), (2, 5), (2, 6),
          (0, 1), (0, 2)]
# subtree sizes (number of nodes in subtree rooted at k, incl. self)
_SUBSIZE = np.ones(K, np.int64)
for _k in range(K - 1, 0, -1):
    _SUBSIZE[(_k - 1) // 2] += _SUBSIZE[_k]


def _np_reference(tokens1, tokens2, parent, level, emb, W_c, b_c,
                  gru_wih_f, gru_whh_f, gru_bih_f, gru_bhh_f,
                  gru_wih_b, gru_whh_b, gru_bih_b, gru_bhh_b, fc_w, fc_b):
    """numpy fallback (used only if the tree-structure inputs are not the
    fixed topology this kernel specializes for)."""
    def sigmoid(x):
        return 1.0 / (1.0 + np.exp(-x))

    def gru_dir(x, w_ih, w_hh, b_ih, b_hh):
        b, s, e = x.shape
        h = np.zeros((b, w_hh.shape[1]), np.float32)
        ys = np.empty((b, s, w_hh.shape[1]), np.float32)
        for t in range(s):
            gi = x[:, t] @ w_ih.T + b_ih
            gh = h @ w_hh.T + b_hh
            ir, iz, inn = np.split(gi, 3, axis=1)
            hr, hz, hn = np.split(gh, 3, axis=1)
            r = sigmoid(ir + hr)
            z = sigmoid(iz + hz)
            n = np.tanh(inn + r * hn)
            h = (1.0 - z) * n + z * h
            ys[:, t] = h
        return ys

    def encode(tokens):
        h = emb[tokens] @ W_c.T + b_c
        for d in range(MAX_DEPTH - 1, 0, -1):
            contrib = np.where((level == d)[:, None], h, 0.0)
            np.add.at(h, parent, contrib)
        enc = np.maximum(h.reshape(B, S, K, E).max(axis=2), 0.0)
        fwd = gru_dir(enc, gru_wih_f, gru_whh_f, gru_bih_f, gru_bhh_f)
        bwd = gru_dir(enc[:, ::-1], gru_wih_b, gru_whh_b, gru_bih_b, gru_bhh_b)[:, ::-1]
        return np.concatenate([fwd, bwd], axis=-1).max(axis=1)

    lvec = encode(tokens1)
    rvec = encode(tokens2)
    y = np.concatenate([lvec, rvec], axis=1) @ fc_w.T + fc_b
    y = y - y.max(axis=1, keepdims=True)
    ey = np.exp(y)
    return (ey / ey.sum(axis=1, keepdims=True)).astype(np.float32)


def _build_program(bias_vecs):
    """Build the 8-core SPMD Bass program. bias_vecs: host fp32 arrays
    (bc_m [128,16], gi_bias [128,6], bhh_n [128,2], fc_db float)."""
    import concourse.bacc as bacc
    import concourse.bass as bass
    import concourse.mybir as mybir
    import concourse.tile as tile

    f32 = mybir.dt.float32
    bf16 = mybir.dt.bfloat16
    i32 = mybir.dt.int32
    AL = mybir.AluOpType
    ACT = mybir.ActivationFunctionType

    nc = bacc.Bacc()

    # ---- DRAM tensors ----
    emb_d = nc.dram_tensor("emb", [V, E], f32, kind="ExternalInput")
    idx_d = [nc.dram_tensor(f"idx{s}", [128, NCALL], i32, kind="ExternalInput")
             for s in range(2)]
    wct_d = nc.dram_tensor("wct", [128, 128], bf16, kind="ExternalInput")
    wih_d = nc.dram_tensor("wih", [128, 6 * 128], bf16, kind="ExternalInput")
    whh_d = nc.dram_tensor("whh", [128, 6 * 128], bf16, kind="ExternalInput")
    fcw_d = nc.dram_tensor("fcw", [128, 4 * 2], bf16, kind="ExternalInput")
    bcm_d = nc.dram_tensor("bcm", [128, K], f32, kind="ExternalInput")
    gib_d = nc.dram_tensor("gib", [128, 6], f32, kind="ExternalInput")
    bhn_d = nc.dram_tensor("bhn", [128, 2], f32, kind="ExternalInput")
    out_d = nc.dram_tensor("out", [BL, 2], f32, kind="ExternalOutput")

    with tile.TileContext(nc) as tc:
        with tc.tile_pool(name="const", bufs=1) as cpool, \
             tc.tile_pool(name="gf32", bufs=4) as gf32_pool, \
             tc.tile_pool(name="gbf", bufs=3) as gbf_pool, \
             tc.tile_pool(name="xp", bufs=2) as x_pool, \
             tc.tile_pool(name="encp", bufs=2) as enc_pool, \
             tc.tile_pool(name="gip", bufs=2) as gi_pool, \
             tc.tile_pool(name="grup", bufs=2) as gru_pool, \
             tc.tile_pool(name="stepp", bufs=8) as step_pool, \
             tc.tile_pool(name="psum_big", bufs=2, space="PSUM") as ps_big, \
             tc.tile_pool(name="psum_gru", bufs=4, space="PSUM") as ps_gru, \
             tc.tile_pool(name="dram", bufs=2, space="DRAM") as dram_pool:

            # ---- constants ----
            idx_t = []
            for s in range(2):
                t = cpool.tile([128, NCALL], i32, name=f"idx{s}_t")
                nc.sync.dma_start(t[:], idx_d[s][:])
                idx_t.append(t)
            wct = cpool.tile([128, 128], bf16, name="wct_t")
            nc.sync.dma_start(wct[:], wct_d[:])
            wih = cpool.tile([128, 6 * 128], bf16, name="wih_t")
            nc.sync.dma_start(wih[:], wih_d[:])
            whh = cpool.tile([128, 6 * 128], bf16, name="whh_t")
            nc.sync.dma_start(whh[:], whh_d[:])
            fcw = cpool.tile([128, 8], bf16, name="fcw_t")
            nc.sync.dma_start(fcw[:], fcw_d[:])
            bcm = cpool.tile([128, K], f32, name="bcm_t")
            nc.sync.dma_start(bcm[:], bcm_d[:])
            gib = cpool.tile([128, 6], f32, name="gib_t")
            nc.sync.dma_start(gib[:], gib_d[:])
            bhn = cpool.tile([128, 2], f32, name="bhn_t")
            nc.sync.dma_start(bhn[:], bhn_d[:])

            pooled = []   # per side [100,16] tiles (f progs | b progs)
            encvs = []

            for s in range(2):
                # ---- gather: 128 indirect calls, 8-call groups -> bf16 -> DRAM ----
                gstage = dram_pool.tile([NT, 128], bf16, name=f"gstage{s}")
                GRP = 8
                for grp in range(NCALL // GRP):
                    gf = gf32_pool.tile([128, GRP, 128], f32, tag="gf")
                    for j in range(GRP):
                        i = grp * GRP + j
                        nc.gpsimd.indirect_dma_start(
                            out=gf[:, j, :], out_offset=None, in_=emb_d[:],
                            in_offset=bass.IndirectOffsetOnAxis(
                                ap=idx_t[s][:, i:i + 1], axis=0))
                    gb = gbf_pool.tile([128, GRP, 128], bf16, tag="gb")
                    nc.vector.tensor_copy(gb[:], gf[:])
                    # slot i lands at DRAM row i = grp*GRP*128 + j*128 + p
                    nc.sync.dma_start(
                        gstage[grp * GRP * 128:(grp + 1) * GRP * 128, :]
                        .rearrange("(j p) e -> p j e", p=128),
                        gb[:])

                # ---- transpose to X [E=128, NT] bf16 (cols = k*1024 + prog*128 + stmt) ----
                X = x_pool.tile([128, NT], bf16, tag="X")
                TCH = 4096
                for c in range(NT // TCH):
                    nc.sync.dma_start_transpose(
                        X[:, c * TCH:(c + 1) * TCH],
                        gstage[c * TCH:(c + 1) * TCH, :])

                # ---- tree child-sum in raw-emb space (linear; W_c applied after) ----
                Xk = X.rearrange("p (k n) -> p k n", k=K)
                for (pnode, cnode) in _EDGES:
                    nc.vector.tensor_tensor(
                        out=Xk[:, pnode, :], in0=Xk[:, pnode, :],
                        in1=Xk[:, cnode, :], op=AL.add)

                # ---- W_c + per-tree node max (+ m_k*b_c) + relu -> enc ----
                enc = enc_pool.tile([128, NTREE], f32, tag="enc")
                CH = 512
                for c in range(NT // CH):
                    k = c // 2
                    half = c % 2
                    ps = ps_big.tile([128, CH], f32, tag="wc")
                    nc.tensor.matmul(ps[:], wct[:], X[:, c * CH:(c + 1) * CH],
                                     start=True, stop=True)
                    esl = enc[:, half * CH:(half + 1) * CH]
                    if k == 0:
                        nc.vector.tensor_scalar(
                            out=esl, in0=ps[:], scalar1=bcm[:, 0:1], scalar2=None,
                            op0=AL.add)
                    else:
                        nc.vector.scalar_tensor_tensor(
                            out=esl, in0=ps[:], scalar=bcm[:, k:k + 1], in1=esl,
                            op0=AL.add, op1=AL.max)
                encb = enc_pool.tile([128, NTREE], bf16, tag="encb")
                nc.vector.tensor_scalar(out=encb[:], in0=enc[:], scalar1=0.0,
                                        scalar2=None, op0=AL.max)
                # ---- input-side GRU matmuls -> GI [128, S*48] bf16 ----
                # block t cols: [rf zf rb zb nf nb] x 8 progs; fwd gates hold
                # stmt t, bwd gates hold stmt S-1-t (written time-reversed)
                GI = gi_pool.tile([128, S * 48], bf16, tag="GI")
                GIw = GI.rearrange("p (t g b) -> p t g b", t=S, g=6)
                for g in range(6):
                    is_bwd = g in (2, 3, 5)
                    for half in range(2):
                        ps = ps_big.tile([128, CH], f32, tag="wc")
                        nc.tensor.matmul(
                            ps[:], wih[:, g * 128:(g + 1) * 128],
                            encb[:, half * CH:(half + 1) * CH],
                            start=True, stop=True)
                        psv = ps.rearrange("p (b t) -> p b t", b=4)
                        dst = GIw[:, :, g, half * 4:(half + 1) * 4]
                        if is_bwd:
                            dst = dst[:, ::-1, :]
                        # dst dims [128, t, 4]; psv [128, 4, t] -> align orders
                        nc.scalar.copy(dst.rearrange("p t b -> p b t"), psv[:])
                encvs.append(GIw)

            # ---- bidirectional GRUs: both sides as independent interleaved streams ----
            # h layout [100(128), 16]: cols 0:8 fwd progs, 8:16 bwd progs
            hbufs = []
            for s in range(2):
                hb = [gru_pool.tile([128, 16], bf16, name=f"h{s}_{i}", tag=f"h{s}{i}")
                      for i in range(2)]
                pt = gru_pool.tile([128, 16], bf16, name=f"pool{s}", tag=f"pool{s}")
                # full memsets: rows 100:127 feed matmuls against zero weight
                # columns; garbage NaNs there would poison 0*NaN accumulation
                nc.vector.memset(hb[0][:], 0)
                nc.vector.memset(hb[1][:], 0)
                nc.vector.memset(pt[:], 0)
                hbufs.append(hb)
                pooled.append(pt)

            for t in range(S):
                for s in range(2):
                    GIw = encvs[s]
                    hprev, hnew = hbufs[s][t % 2], hbufs[s][(t + 1) % 2]
                    pool_t = pooled[s]
                    ps = ps_gru.tile([128, 48], f32, tag="gru")
                    # cols: rf 0:8 zf 8:16 rb 16:24 zb 24:32 | ghn_f 32:40 ghn_b 40:48
                    mm = [(0, 0, 0), (1, 8, 0), (2, 16, 8),
                          (3, 24, 8), (4, 32, 0), (5, 40, 8)]
                    for g, col, hc in mm:
                        nc.tensor.matmul(
                            ps[:, col:col + 8], whh[:, g * 128:(g + 1) * 128],
                            hprev[:, hc:hc + 8], start=True, stop=True)
                    srz = step_pool.tile([128, 32], bf16, tag="srz")
                    nc.vector.tensor_tensor(
                        out=srz[0:100, :], in0=ps[0:100, 0:32],
                        in1=GIw[0:100, t, 0:4, :], op=AL.add)
                    rz = step_pool.tile([128, 32], bf16, tag="rz")
                    nc.scalar.activation(rz[0:100, :], srz[0:100, :], ACT.Sigmoid)
                    # n preact: av = gin + r * ghn  (biases zero: fast-path gate)
                    m_ = step_pool.tile([128, 16], bf16, tag="m_")
                    rcols = rz.rearrange("p (h c) -> p h c", h=2)[0:100, :, 0:8]
                    nc.vector.tensor_tensor(out=m_[0:100, :], in0=ps[0:100, 32:48],
                                            in1=rcols, op=AL.mult)
                    av = step_pool.tile([128, 16], bf16, tag="av")
                    nc.vector.tensor_tensor(out=av[0:100, :], in0=m_[0:100, :],
                                            in1=GIw[0:100, t, 4:6, :], op=AL.add)
                    nt_ = step_pool.tile([128, 16], bf16, tag="nt")
                    nc.scalar.activation(nt_[0:100, :], av[0:100, :], ACT.Tanh)
                    # h' = h + z'*(n - h)   (z' = 1-z via negated z weights)
                    dd = step_pool.tile([128, 16], bf16, tag="dd")
                    nc.vector.tensor_tensor(out=dd[0:100, :], in0=nt_[0:100, :],
                                            in1=hprev[0:100, :], op=AL.subtract)
                    zcols = rz.rearrange("p (h c) -> p h c", h=2)[0:100, :, 8:16]
                    ee = step_pool.tile([128, 16], bf16, tag="ee")
                    nc.vector.tensor_tensor(out=ee[0:100, :], in0=dd[0:100, :],
                                            in1=zcols, op=AL.mult)
                    nc.vector.tensor_tensor(out=hnew[0:100, :], in0=hprev[0:100, :],
                                            in1=ee[0:100, :], op=AL.add)
                    if t == 0:
                        nc.vector.tensor_copy(pool_t[0:100, :], hnew[0:100, :])
                    else:
                        nc.vector.tensor_tensor(
                            out=pool_t[0:100, :], in0=pool_t[0:100, :],
                            in1=hnew[0:100, :], op=AL.max)

            # ---- fc + softmax ----
            # vec chunks: 0=fwd_L 1=bwd_L 2=fwd_R 3=bwd_R ; pooled[s] cols 0:8 f, 8:16 b
            psf = ps_gru.tile([128, 8], f32, tag="fc", bufs=1)
            chunks = [(0, 0), (0, 8), (1, 0), (1, 8)]
            for ci, (sd, col) in enumerate(chunks):
                nc.tensor.matmul(
                    psf[0:2, :], fcw[:, ci * 2:(ci + 1) * 2],
                    pooled[sd][:, col:col + 8],
                    start=(ci == 0), stop=(ci == 3))
            t32 = step_pool.tile([128, 32], f32, tag="t32")
            nc.vector.memset(t32[0:32, :], 0)
            nc.vector.tensor_copy(t32[0:2, 0:8], psf[0:2, :])
            t32b = step_pool.tile([128, 32], f32, tag="t32b")
            nc.vector.transpose(t32b[0:32, :], t32[0:32, :])
            dcol = step_pool.tile([128, 2], f32, tag="dcol")
            nc.vector.tensor_tensor(out=dcol[0:8, 0:1], in0=t32b[0:8, 0:1],
                                    in1=t32b[0:8, 1:2], op=AL.subtract)
            outt = step_pool.tile([128, 2], f32, tag="outt")
            nc.scalar.activation(outt[0:8, 0:1], dcol[0:8, 0:1], ACT.Sigmoid,
                                 bias=float(bias_vecs["fc_db"]))
            nc.vector.tensor_scalar(
                out=outt[0:8, 1:2], in0=outt[0:8, 0:1], scalar1=-1.0, scalar2=1.0,
                op0=AL.mult, op1=AL.add)
            nc.sync.dma_start(out_d[:], outt[0:8, 0:2])

    nc.compile()
    return nc


_CACHED = {}


def kernel(**inputs):
    inputs = {k: np.asarray(v) for k, v in inputs.items()}
    tokens1 = inputs["tokens1"].astype(np.int64)
    tokens2 = inputs["tokens2"].astype(np.int64)
    parent = inputs["parent"].astype(np.int64)
    level = inputs["level"].astype(np.int64)
    emb = inputs["emb"].astype(np.float32)
    W_c = inputs["W_c"].astype(np.float32)
    b_c = inputs["b_c"].astype(np.float32)
    fc_w = inputs["fc_w"].astype(np.float32)
    fc_b = inputs["fc_b"].astype(np.float32)
    gw = {k: inputs[k].astype(np.float32) for k in (
        "gru_wih_f", "gru_whh_f", "gru_bih_f", "gru_bhh_f",
        "gru_wih_b", "gru_whh_b", "gru_bih_b", "gru_bhh_b")}

    # verify the fixed tree topology this kernel specializes for
    base = np.arange(B * S, dtype=np.int64)[:, None] * K
    exp_parent = (base + _LOCAL_PARENT[None, :]).reshape(-1)
    exp_level = np.tile(_LOCAL_LEVEL, B * S)
    if not (np.array_equal(parent, exp_parent) and np.array_equal(level, exp_level)):
        return _np_reference(tokens1, tokens2, parent, level, emb, W_c, b_c,
                             gw["gru_wih_f"], gw["gru_whh_f"], gw["gru_bih_f"],
                             gw["gru_bhh_f"], gw["gru_wih_b"], gw["gru_whh_b"],
                             gw["gru_bih_b"], gw["gru_bhh_b"], fc_w, fc_b)

    # ---- host-side weight packing (layout prep only) ----
    bf = ml_dtypes.bfloat16
    wct = np.ascontiguousarray(W_c.T).astype(bf)                   # [128,128] lhsT
    # gate order: 0=(f,r) 1=(f,z) 2=(b,r) 3=(b,z) 4=(f,n) 5=(b,n); z negated
    def pack_w(w, negate):  # w [100, D] -> [D, 128] lhsT padded
        out = np.zeros((w.shape[1], 128), np.float32)
        out[:, :100] = w.T * (-1.0 if negate else 1.0)
        return out
    gates = [("f", 0, False), ("f", 1, True), ("b", 0, False),
             ("b", 1, True), ("f", 2, False), ("b", 2, False)]
    wih = np.concatenate(
        [pack_w(gw[f"gru_wih_{d}"][gi * H:(gi + 1) * H], neg)
         for d, gi, neg in gates], axis=1).astype(bf)               # [128, 6*128]
    whh_full = np.concatenate(
        [pack_w(gw[f"gru_whh_{d}"][gi * H:(gi + 1) * H], neg)
         for d, gi, neg in gates], axis=1)                          # [100, 6*128]
    whh = np.zeros((128, 6 * 128), np.float32)
    whh[:H] = whh_full
    whh = whh.astype(bf)
    fcw = np.zeros((128, 8), np.float32)
    for ci in range(4):                                            # chunks of 100
        fcw[:H, ci * 2:(ci + 1) * 2] = fc_w[:, ci * H:(ci + 1) * H].T
    fcw = fcw.astype(bf)
    # bias vectors (general path; zeros in practice)
    bcm = np.zeros((128, K), np.float32)
    bcm[:E] = b_c[:, None] * _SUBSIZE[None, :]
    gib = np.zeros((128, 6), np.float32)
    for g, (d, gi, neg) in enumerate(gates):
        bsum = gw[f"gru_bih_{d}"][gi * H:(gi + 1) * H].copy()
        if gi != 2:
            bsum = bsum + gw[f"gru_bhh_{d}"][gi * H:(gi + 1) * H]
        gib[:H, g] = bsum * (-1.0 if neg else 1.0)
    bhn = np.zeros((128, 2), np.float32)
    bhn[:H, 0] = gw["gru_bhh_f"][2 * H:3 * H]
    bhn[:H, 1] = gw["gru_bhh_b"][2 * H:3 * H]
    # NOTE: bhn is used as one per-partition scalar for both dirs (col 0);
    # if bhh_n differs between dirs and is nonzero we must fall back.
    bias_vecs = {"fc_db": float(fc_b[0] - fc_b[1])}
    # fast path folds the input-side GRU matmuls into per-step PSUM with no
    # bias injection; nonzero GRU biases take the host fallback
    zero_bias = all(
        not gw[k].any() for k in ("gru_bih_f", "gru_bhh_f", "gru_bih_b", "gru_bhh_b"))
    if not zero_bias or not np.array_equal(bhn[:, 0], bhn[:, 1]):
        return _np_reference(tokens1, tokens2, parent, level, emb, W_c, b_c,
                             gw["gru_wih_f"], gw["gru_whh_f"], gw["gru_bih_f"],
                             gw["gru_bhh_f"], gw["gru_wih_b"], gw["gru_whh_b"],
                             gw["gru_bih_b"], gw["gru_bhh_b"], fc_w, fc_b)

    # ---- per-core token index arrays, (k, prog, stmt) order ----
    def idx_for(tokens, core):
        t3 = tokens.reshape(B, S, K)[core * BL:(core + 1) * BL]    # [8,128,16]
        tk = np.transpose(t3, (2, 0, 1)).reshape(-1)               # k-outer flat [16384]
        # call i covers flat positions [128i, 128(i+1)); partition p = pos%128
        return tk.reshape(NCALL, 128).T.astype(np.int32).copy()    # [128, NCALL]

    from concourse.bass_utils import run_bass_kernel_spmd

    key = ("prog", bias_vecs["fc_db"])
    if key not in _CACHED:
        _CACHED[key] = _build_program(bias_vecs)
    nc = _CACHED[key]

    in_maps = []
    for c in range(NCORES):
        in_maps.append({
            "emb": emb,
            "idx0": idx_for(tokens1, c),
            "idx1": idx_for(tokens2, c),
            "wct": wct, "wih": wih, "whh": whh, "fcw": fcw,
            "bcm": bcm, "gib": gib, "bhn": np.ascontiguousarray(bhn[:, 0:2]),
        })

    if os.environ.get("BPCC_SIM"):
        # debug path: CoreSim core 0 only; rows 8: are invalid
        from concourse.bass_interp import CoreSim
        sim = CoreSim(nc)
        for k, v in in_maps[0].items():
            sim.tensor(k)[:] = v
        sim.simulate()
        o0 = np.asarray(sim.tensor("out")).copy()
        return np.vstack([o0] * NCORES).astype(np.float32)

    trace = bool(os.environ.get("BPCC_TRACE"))
    if trace:
        try:
            import axon_prof_shim  # noqa: F401
        except ImportError:
            trace = False
    res = run_bass_kernel_spmd(nc, in_maps, core_ids=list(range(NCORES)),
                               trace=trace)
    if trace and res.exec_time_ns:
        print(f"HW exec time: {res.exec_time_ns} ns")
    out = np.vstack([res.results[c]["out"] for c in range(NCORES)])
    return out.astype(np.float32)
